# revision 1
# baseline (speedup 1.0000x reference)
"""Trainium2 Bass kernel for ConeProjection.

Math (per batch element b):
    vn    = v / max(||v||, 1e-14)
    W     = [R[:,0], R[:,1], t - eyes]          (3 rows)
    u_a   = vn . W_a
    G_ac  = W_a . W_c
    sigma = u u^T - alpha * G                   (symmetric 3x3)
    out[k] = P[k] sigma P[k]^T ,  P = 13x13 grid, 169 points
           = s . q[k]
    s = [s00, s11, s22, s01, s02, s12]   (sigma entries)
    q[k] = [x^2, y^2, 1, 2xy, 2x, 2y]    (x,y = P[k,0], P[k,1])

Strategy: pure data-parallel over 8 NeuronCores (batch 131072 -> 16384/core).
Per core, partition p holds batch [p*128, (p+1)*128); within-partition index i.
DVE computes the 6 sigma entries into an AoS tile S[p, 6*i+c]; PE transposes
S chunks to get S^T tiles (components on partitions) which become matmul
stationary weights against a constant block-diagonal Q (3 batch rows per
matmul, K=18, N=507), producing out rows [128, 507] directly in batch-major
layout for contiguous DMA stores.
"""

from contextlib import ExitStack

import numpy as np

import concourse.bass as bass
import concourse.bacc as bacc
import concourse.tile as tile
from concourse import mybir
from concourse.bass_utils import run_bass_kernel_spmd

N_CORES = 8
B = 131072
BC = B // N_CORES          # 16384 per core
P = 128                    # partitions
NI = BC // P               # 128 within-partition batch indices
KG = 169                   # grid points
F32 = mybir.dt.float32
F32R = mybir.dt.float32r

# i's are processed in groups of 3 (one matmul per group, K=18, N=507);
# 3 groups per PE-transpose block (output partitions 0/32/64 -> matmul
# base-partition constraint).
GROUP = 3
N_FULL_GROUPS = NI // GROUP        # 42
REM_I = NI - GROUP * N_FULL_GROUPS  # 2
BLOCK_GROUPS = 3
N_BLOCKS = N_FULL_GROUPS // BLOCK_GROUPS  # 14
assert N_FULL_GROUPS % BLOCK_GROUPS == 0
PADC = 6 * NI              # S_aos columns

# tuning knobs (read at build time)
USE_F32R = False           # fp32r (fast, ~1e-4 rel err) vs fp32 matmuls
COPY_MOD = 2               # 1 of COPY_MOD output copies goes to ACT (0: all DVE)
OUT_DMA_ON_ACT = False     # issue output DMAs from the ACT HWDGE ring
PACK_ON_ACT = True         # transpose pack-copy engine
SQUARES_ON_ACT = True      # self-dot muls (x*x) via ACT Square
ST_ON_ACT = True           # S^T PSUM->SBUF copies on ACT
CHUNK_BLOCKS = 7           # blocks per elementwise chunk (7 -> 2 chunks)
PSM_BUFS = 6               # PSUM matmul-out banks
STAGE_BUFS = 4             # SBUF output staging buffers
ST_BUFS = 4                # S^T SBUF buffers


def _grid_q():
    ii, jj = np.meshgrid(np.arange(13), np.arange(13), indexing="ij")
    x = ((ii - 6) / 6.0).reshape(-1)
    y = ((jj - 6) / 6.0).reshape(-1)
    q = np.stack([x * x, y * y, np.ones(KG), 2 * x * y, 2 * x, 2 * y], axis=0)
    return q.astype(np.float32)  # [6, 169]


def _q_blockdiag(q, m):
    out = np.zeros((6 * m, KG * m), np.float32)
    for a in range(m):
        out[6 * a : 6 * a + 6, KG * a : KG * a + KG] = q
    return out


def make_q96():
    """[96, 508]: K=18 block-diag Q replicated at partition bases 0/32/64.

    Padded to 508 columns: fp32r matmuls need an even moving free dim.
    """
    q18 = _q_blockdiag(_grid_q(), 3)  # [18, 507]
    out = np.zeros((96, 508), np.float32)
    for g in range(3):
        out[32 * g : 32 * g + 18, 0:507] = q18
    return out


def make_q12():
    return _q_blockdiag(_grid_q(), 2)  # [12, 338]


def build_nc(reps: int = 1, elem_chunks: int = 1, loop_n: int = 0):
    nc = bacc.Bacc("TRN2", target_bir_lowering=False, debug=False,
                   num_devices=N_CORES)

    eyes_d = nc.declare_dram_parameter("eyes", [BC, 3], F32, isOutput=False)
    v_d = nc.declare_dram_parameter("v", [BC, 3], F32, isOutput=False)
    r_d = nc.declare_dram_parameter("R", [BC, 3, 3], F32, isOutput=False)
    t_d = nc.declare_dram_parameter("t", [BC, 3], F32, isOutput=False)
    a_d = nc.declare_dram_parameter("alpha", [BC], F32, isOutput=False)
    q96_d = nc.declare_dram_parameter("q96", [96, 508], F32, isOutput=False)
    q12_d = nc.declare_dram_parameter("q12", [12, 338], F32, isOutput=False)
    id_d = nc.declare_dram_parameter("ident", [P, P], F32, isOutput=False)
    out_d = nc.declare_dram_parameter("out", [BC, KG], F32, isOutput=True)

    with tile.TileContext(nc) as tc:
        with ExitStack() as ctx:
            const = ctx.enter_context(tc.tile_pool(name="const", bufs=1))
            q96_f = const.tile([96, 508], F32)
            nc.sync.dma_start(q96_f[:], q96_d.ap())
            q12_f = const.tile([12, 338], F32)
            nc.sync.dma_start(q12_f[:], q12_d.ap())
            if USE_F32R:
                q96_sb = const.tile([96, 508], F32R)
                nc.vector.tensor_copy(q96_sb[:], q96_f[:])
                q12_sb = const.tile([12, 338], F32R)
                nc.vector.tensor_copy(q12_sb[:], q12_f[:])
            else:
                q96_sb, q12_sb = q96_f, q12_f
            id_sb = const.tile([P, P], F32)
            nc.sync.dma_start(id_sb[:], id_d.ap())

            pools = dict(
                io=ctx.enter_context(tc.tile_pool(name="io", bufs=2)),
                scr=ctx.enter_context(tc.tile_pool(name="scr", bufs=1)),
                tmpp=ctx.enter_context(tc.tile_pool(name="tmp", bufs=2)),
                stp=ctx.enter_context(tc.tile_pool(name="st", bufs=ST_BUFS)),
                stagep=ctx.enter_context(tc.tile_pool(name="stage", bufs=STAGE_BUFS)),
                pkp=ctx.enter_context(tc.tile_pool(name="pk", bufs=3)),
                psq=ctx.enter_context(tc.tile_pool(name="psq", bufs=2, space="PSUM")),
                psm=ctx.enter_context(tc.tile_pool(name="psm", bufs=PSM_BUFS, space="PSUM")),
            )
            if loop_n:
                with tc.For_i(0, loop_n, 1):
                    _emit_one_pass(nc, tc, pools, elem_chunks,
                                   eyes_d, v_d, r_d, t_d, a_d, out_d,
                                   q96_sb, q12_sb, id_sb)
            else:
                for rep in range(reps):
                    _emit_one_pass(nc, tc, pools, elem_chunks,
                                   eyes_d, v_d, r_d, t_d, a_d, out_d,
                                   q96_sb, q12_sb, id_sb)

    nc.compile()
    return nc


def _emit_one_pass(nc, tc, pools, elem_chunks,
                   eyes_d, v_d, r_d, t_d, a_d, out_d,
                   q96_sb, q12_sb, id_sb):
    X = mybir.AxisListType.X
    ADD = mybir.AluOpType.add

    io = pools["io"]
    scr = pools["scr"]
    tmpp = pools["tmpp"]
    stp = pools["stp"]
    stagep = pools["stagep"]
    pkp = pools["pkp"]
    psq = pools["psq"]
    psm = pools["psm"]

    # DRAM views (per-partition contiguous)
    eyes_f = eyes_d.ap().rearrange("(p i) c -> p (i c)", p=P)
    v_f = v_d.ap().rearrange("(p i) c -> p (i c)", p=P)
    r_f = r_d.ap().rearrange("(p i) a b -> p (i a b)", p=P)
    t_f = t_d.ap().rearrange("(p i) c -> p (i c)", p=P)
    out_flat = out_d.ap().rearrange("(p i) k -> p (i k)", p=P)  # [P, NI*KG]

    a_sb = io.tile([P, NI], F32)
    nc.sync.dma_start(a_sb[:], a_d.ap().rearrange("(p i) -> p i", p=P))

    # chunks: (i0, i1, blk0, blk1, has_rem)
    chunks = []
    b0 = 0
    while b0 < N_BLOCKS:
        b1 = min(b0 + CHUNK_BLOCKS, N_BLOCKS)
        last = b1 == N_BLOCKS
        chunks.append((9 * b0, NI if last else 9 * b1, b0, b1, last))
        b0 = b1
    copy_idx = 0
    for ci, (i0, i1, blk0, blk1, has_rem) in enumerate(chunks):
        ni = i1 - i0

        eyes_sb = io.tile([P, 3 * ni], F32, tag=f"eyes{ci}")
        nc.sync.dma_start(eyes_sb[:], eyes_f[:, 3 * i0 : 3 * i1])
        v_sb = io.tile([P, 3 * ni], F32, tag=f"v{ci}")
        nc.sync.dma_start(v_sb[:], v_f[:, 3 * i0 : 3 * i1])
        r_sb = io.tile([P, 9 * ni], F32, tag=f"r{ci}")
        nc.sync.dma_start(r_sb[:], r_f[:, 9 * i0 : 9 * i1])
        t_sb = io.tile([P, 3 * ni], F32, tag=f"t{ci}")
        nc.sync.dma_start(t_sb[:], t_f[:, 3 * i0 : 3 * i1])

        w2_sb = scr.tile([P, 3 * ni], F32, tag=f"w2_{ci}")
        d_aos = scr.tile([P, 3 * ni], F32, tag=f"d_{ci}")
        u_aos = scr.tile([P, 3 * ni], F32, tag=f"u_{ci}")
        g_aos = scr.tile([P, 6 * ni], F32, tag=f"g_{ci}")
        ag = scr.tile([P, 6 * ni], F32, tag=f"ag_{ci}")
        s_aos = scr.tile([P, 6 * ni + 16], F32, tag=f"s_{ci}")
        nv2 = scr.tile([P, ni], F32, tag=f"nv2_{ci}")
        rinv = scr.tile([P, ni], F32, tag=f"rinv_{ci}")

        v3 = v_sb[:].rearrange("p (i c) -> p i c", c=3)
        w23 = w2_sb[:].rearrange("p (i c) -> p i c", c=3)
        r9 = r_sb[:].rearrange("p (i a b) -> p i a b", a=3, b=3)
        d3 = d_aos[:].rearrange("p (i a) -> p i a", a=3)
        u3 = u_aos[:].rearrange("p (i a) -> p i a", a=3)
        g6 = g_aos[:].rearrange("p (i g) -> p i g", g=6)
        ag6 = ag[:].rearrange("p (i g) -> p i g", g=6)
        s6 = s_aos[:, 0 : 6 * ni].rearrange("p (i c) -> p i c", c=6)
        w0 = r9[:, :, :, 0]
        w1 = r9[:, :, :, 1]

        nc.vector.tensor_sub(w2_sb[:], t_sb[:], eyes_sb[:])

        dots = [
            (v3, v3, nv2[:]),
            (v3, w0, d3[:, :, 0]),
            (v3, w1, d3[:, :, 1]),
            (v3, w23, d3[:, :, 2]),
            (w0, w0, g6[:, :, 0]),
            (w1, w1, g6[:, :, 1]),
            (w23, w23, g6[:, :, 2]),
            (w0, w1, g6[:, :, 3]),
            (w0, w23, g6[:, :, 4]),
            (w1, w23, g6[:, :, 5]),
        ]
        for a_v, b_v, o_v in dots:
            tmp = tmpp.tile([P, 3 * ni], F32, tag=f"dm{ci}")
            t3 = tmp[:].rearrange("p (i c) -> p i c", c=3)
            if SQUARES_ON_ACT and a_v is b_v:
                nc.scalar.square(t3, a_v)
            else:
                nc.vector.tensor_mul(t3, a_v, b_v)
            nc.vector.tensor_reduce(o_v, t3, axis=X, op=ADD)

        nrm = tmpp.tile([P, ni], F32, tag=f"nrm{ci}")
        nc.scalar.sqrt(nrm[:], nv2[:])
        nc.vector.tensor_scalar_max(nrm[:], nrm[:], 1e-14)
        nc.vector.reciprocal(rinv[:], nrm[:])

        r_b3 = rinv[:].unsqueeze(2).broadcast_to((P, ni, 3))
        nc.vector.tensor_mul(u3, d3, r_b3)

        nc.vector.tensor_mul(s6[:, :, 0:3], u3, u3)
        nc.vector.tensor_mul(s6[:, :, 3:6:2], u3[:, :, 0:2], u3[:, :, 1:3])
        nc.vector.tensor_mul(s6[:, :, 4], u3[:, :, 0], u3[:, :, 2])

        a_b6 = a_sb[:, i0:i1].unsqueeze(2).broadcast_to((P, ni, 6))
        nc.vector.tensor_mul(ag6, g6, a_b6)
        nc.vector.tensor_sub(s6, s6, ag6)

        s_h = s_aos[:].tensor
        s_w = s_aos[:].ap[0][0]  # per-partition alloc width (elements)
        for b in range(blk0, blk1):
            lo = 54 * b - 6 * i0  # local col offset in this chunk's s_aos
            pack96 = pkp.tile([P, 96], F32, tag="pk")
            pack_dst = pack96[:].rearrange("p (g c) -> p g c", c=32)
            pack_src = bass.AP(s_h, lo, [[s_w, P], [18, 3], [1, 32]])
            if PACK_ON_ACT:
                nc.scalar.copy(pack_dst, pack_src)
            else:
                nc.vector.tensor_copy(pack_dst, pack_src)
            st_ps = psq.tile([96, P], F32)
            nc.tensor.transpose(st_ps[:], pack96[:], id_sb[:])
            st_sb = stp.tile([96, P], F32R if USE_F32R else F32)
            if ST_ON_ACT:
                nc.scalar.copy(st_sb[:], st_ps[:])
            else:
                nc.vector.tensor_copy(st_sb[:], st_ps[:])

            stage = stagep.tile([P, 3 * 507], F32, tag="stage")
            for g in range(BLOCK_GROUPS):
                o_ps = psm.tile([P, 508], F32, tag="mmout")
                nc.tensor.matmul(
                    o_ps[:],
                    st_sb[32 * g : 32 * g + 18, :],
                    q96_sb[32 * g : 32 * g + 18, :],
                    start=True,
                    stop=True,
                )
                dst = stage[:, 507 * g : 507 * (g + 1)]
                if COPY_MOD and copy_idx % COPY_MOD == COPY_MOD - 1:
                    nc.scalar.copy(dst, o_ps[:, 0:507])
                else:
                    nc.vector.tensor_copy(dst, o_ps[:, 0:507])
                copy_idx += 1
            out_dma_eng = nc.scalar if OUT_DMA_ON_ACT else nc.sync
            out_dma_eng.dma_start(
                out_flat[:, 1521 * b : 1521 * (b + 1)], stage[:]
            )

        if has_rem and REM_I:
            lo = 54 * N_BLOCKS - 6 * i0
            st_ps = psq.tile([96, P], F32)
            nc.tensor.transpose(
                st_ps[0 : 6 * REM_I, :], s_aos[:, lo : lo + 6 * REM_I], id_sb[:]
            )
            st_sb = stp.tile([96, P], F32R if USE_F32R else F32)
            nc.vector.tensor_copy(st_sb[0 : 6 * REM_I, :], st_ps[0 : 6 * REM_I, :])
            o_ps = psm.tile([P, 508], F32, tag="mmout")
            nc.tensor.matmul(
                o_ps[:, 0 : KG * REM_I],
                st_sb[0 : 6 * REM_I, :],
                q12_sb[:],
                start=True,
                stop=True,
            )
            stage = stagep.tile([P, 3 * 507], F32, tag="stage")
            nc.vector.tensor_copy(stage[:, 0 : KG * REM_I], o_ps[:, 0 : KG * REM_I])
            nc.sync.dma_start(
                out_flat[:, 1521 * N_BLOCKS :], stage[:, 0 : KG * REM_I]
            )


_NC_CACHE = {}


def _get_nc(reps=1):
    if reps not in _NC_CACHE:
        _NC_CACHE[reps] = build_nc(reps)
    return _NC_CACHE[reps]


def make_in_maps(eyes, v, R, t, alpha):
    q96 = make_q96()
    q12 = make_q12()
    ident = np.eye(P, dtype=np.float32)
    eyes = np.ascontiguousarray(eyes, np.float32).reshape(N_CORES, BC, 3)
    v = np.ascontiguousarray(v, np.float32).reshape(N_CORES, BC, 3)
    R = np.ascontiguousarray(R, np.float32).reshape(N_CORES, BC, 3, 3)
    t = np.ascontiguousarray(t, np.float32).reshape(N_CORES, BC, 3)
    alpha = np.ascontiguousarray(alpha, np.float32).reshape(N_CORES, BC)
    return [
        {
            "eyes": eyes[c], "v": v[c], "R": R[c], "t": t[c], "alpha": alpha[c],
            "q96": q96, "q12": q12, "ident": ident,
        }
        for c in range(N_CORES)
    ]


def kernel(eyes, v, R, t, alpha):
    nc = _get_nc(1)
    in_maps = make_in_maps(eyes, v, R, t, alpha)
    res = run_bass_kernel_spmd(nc, in_maps, list(range(N_CORES)))
    out = np.concatenate([res.results[c]["out"] for c in range(N_CORES)], axis=0)
    return out.astype(np.float32, copy=False)



# revision 2
# speedup vs baseline: 1.1279x; 1.1279x over previous
"""Trainium2 Bass kernel for ConeProjection.

Math (per batch element b):
    vn    = v / max(||v||, 1e-14)
    W     = [R[:,0], R[:,1], t - eyes]          (3 rows)
    u_a   = vn . W_a
    G_ac  = W_a . W_c
    sigma = u u^T - alpha * G                   (symmetric 3x3)
    out[k] = P[k] sigma P[k]^T ,  P = 13x13 grid, 169 points
           = s . q[k]
    s = [s00, s11, s22, s01, s02, s12]   (sigma entries)
    q[k] = [x^2, y^2, 1, 2xy, 2x, 2y]    (x,y = P[k,0], P[k,1])

Strategy: pure data-parallel over 8 NeuronCores (batch 131072 -> 16384/core).
Per core, partition p holds batch [p*128, (p+1)*128); within-partition index i.
DVE computes the 6 sigma entries into an AoS tile S[p, 6*i+c]; PE transposes
S chunks to get S^T tiles (components on partitions) which become matmul
stationary weights against a constant block-diagonal Q (3 batch rows per
matmul, K=18, N=507), producing out rows [128, 507] directly in batch-major
layout for contiguous DMA stores.
"""

from contextlib import ExitStack

import numpy as np

import concourse.bass as bass
import concourse.bacc as bacc
import concourse.tile as tile
from concourse import mybir
from concourse.bass_utils import run_bass_kernel_spmd

N_CORES = 8
B = 131072
BC = B // N_CORES          # 16384 per core
P = 128                    # partitions
NI = BC // P               # 128 within-partition batch indices
KG = 169                   # grid points
F32 = mybir.dt.float32
F32R = mybir.dt.float32r

# i's are processed in groups of 3 (one matmul per group, K=18, N=507);
# 3 groups per PE-transpose block (output partitions 0/32/64 -> matmul
# base-partition constraint).
GROUP = 3
N_FULL_GROUPS = NI // GROUP        # 42
REM_I = NI - GROUP * N_FULL_GROUPS  # 2
BLOCK_GROUPS = 3
N_BLOCKS = N_FULL_GROUPS // BLOCK_GROUPS  # 14
assert N_FULL_GROUPS % BLOCK_GROUPS == 0
PADC = 6 * NI              # S_aos columns

# tuning knobs (read at build time)
USE_F32R = True            # fp32r (fast, ~1e-4 rel err) vs fp32 matmuls
COPY_MOD = 2               # 1 of COPY_MOD output copies goes to ACT (0: all DVE)
OUT_DMA_ON_ACT = False     # issue output DMAs from the ACT HWDGE ring
PACK_ON_ACT = True         # transpose pack-copy engine
SQUARES_ON_ACT = True      # self-dot muls (x*x) via ACT Square
ST_ON_ACT = True           # S^T PSUM->SBUF copies on ACT
CHUNK_BLOCKS = 7           # blocks per elementwise chunk (7 -> 2 chunks)
PSM_BUFS = 6               # PSUM matmul-out banks
STAGE_BUFS = 4             # SBUF output staging buffers
ST_BUFS = 4                # S^T SBUF buffers


def _grid_q():
    ii, jj = np.meshgrid(np.arange(13), np.arange(13), indexing="ij")
    x = ((ii - 6) / 6.0).reshape(-1)
    y = ((jj - 6) / 6.0).reshape(-1)
    q = np.stack([x * x, y * y, np.ones(KG), 2 * x * y, 2 * x, 2 * y], axis=0)
    return q.astype(np.float32)  # [6, 169]


def _q_blockdiag(q, m):
    out = np.zeros((6 * m, KG * m), np.float32)
    for a in range(m):
        out[6 * a : 6 * a + 6, KG * a : KG * a + KG] = q
    return out


def make_q96():
    """[96, 508]: K=18 block-diag Q replicated at partition bases 0/32/64.

    Padded to 508 columns: fp32r matmuls need an even moving free dim.
    """
    q18 = _q_blockdiag(_grid_q(), 3)  # [18, 507]
    out = np.zeros((96, 508), np.float32)
    for g in range(3):
        out[32 * g : 32 * g + 18, 0:507] = q18
    return out


def make_q12():
    return _q_blockdiag(_grid_q(), 2)  # [12, 338]


def build_nc(reps: int = 1, elem_chunks: int = 1, loop_n: int = 0):
    nc = bacc.Bacc("TRN2", target_bir_lowering=False, debug=False,
                   num_devices=N_CORES)

    eyes_d = nc.declare_dram_parameter("eyes", [BC, 3], F32, isOutput=False)
    v_d = nc.declare_dram_parameter("v", [BC, 3], F32, isOutput=False)
    r_d = nc.declare_dram_parameter("R", [BC, 3, 3], F32, isOutput=False)
    t_d = nc.declare_dram_parameter("t", [BC, 3], F32, isOutput=False)
    a_d = nc.declare_dram_parameter("alpha", [BC], F32, isOutput=False)
    q96_d = nc.declare_dram_parameter("q96", [96, 508], F32, isOutput=False)
    q12_d = nc.declare_dram_parameter("q12", [12, 338], F32, isOutput=False)
    id_d = nc.declare_dram_parameter("ident", [P, P], F32, isOutput=False)
    out_d = nc.declare_dram_parameter("out", [BC, KG], F32, isOutput=True)

    with tile.TileContext(nc) as tc:
        with ExitStack() as ctx:
            const = ctx.enter_context(tc.tile_pool(name="const", bufs=1))
            q96_f = const.tile([96, 508], F32)
            nc.sync.dma_start(q96_f[:], q96_d.ap())
            q12_f = const.tile([12, 338], F32)
            nc.sync.dma_start(q12_f[:], q12_d.ap())
            if USE_F32R:
                q96_sb = const.tile([96, 508], F32R)
                nc.vector.tensor_copy(q96_sb[:], q96_f[:])
                q12_sb = const.tile([12, 338], F32R)
                nc.vector.tensor_copy(q12_sb[:], q12_f[:])
            else:
                q96_sb, q12_sb = q96_f, q12_f
            id_sb = const.tile([P, P], F32)
            nc.sync.dma_start(id_sb[:], id_d.ap())

            pools = dict(
                io=ctx.enter_context(tc.tile_pool(name="io", bufs=2)),
                scr=ctx.enter_context(tc.tile_pool(name="scr", bufs=1)),
                tmpp=ctx.enter_context(tc.tile_pool(name="tmp", bufs=2)),
                stp=ctx.enter_context(tc.tile_pool(name="st", bufs=ST_BUFS)),
                stagep=ctx.enter_context(tc.tile_pool(name="stage", bufs=STAGE_BUFS)),
                pkp=ctx.enter_context(tc.tile_pool(name="pk", bufs=3)),
                psq=ctx.enter_context(tc.tile_pool(name="psq", bufs=2, space="PSUM")),
                psm=ctx.enter_context(tc.tile_pool(name="psm", bufs=PSM_BUFS, space="PSUM")),
            )
            if loop_n:
                with tc.For_i(0, loop_n, 1):
                    _emit_one_pass(nc, tc, pools, elem_chunks,
                                   eyes_d, v_d, r_d, t_d, a_d, out_d,
                                   q96_sb, q12_sb, id_sb)
            else:
                for rep in range(reps):
                    _emit_one_pass(nc, tc, pools, elem_chunks,
                                   eyes_d, v_d, r_d, t_d, a_d, out_d,
                                   q96_sb, q12_sb, id_sb)

    nc.compile()
    return nc


def _emit_one_pass(nc, tc, pools, elem_chunks,
                   eyes_d, v_d, r_d, t_d, a_d, out_d,
                   q96_sb, q12_sb, id_sb):
    X = mybir.AxisListType.X
    ADD = mybir.AluOpType.add

    io = pools["io"]
    scr = pools["scr"]
    tmpp = pools["tmpp"]
    stp = pools["stp"]
    stagep = pools["stagep"]
    pkp = pools["pkp"]
    psq = pools["psq"]
    psm = pools["psm"]

    # DRAM views (per-partition contiguous)
    eyes_f = eyes_d.ap().rearrange("(p i) c -> p (i c)", p=P)
    v_f = v_d.ap().rearrange("(p i) c -> p (i c)", p=P)
    r_f = r_d.ap().rearrange("(p i) a b -> p (i a b)", p=P)
    t_f = t_d.ap().rearrange("(p i) c -> p (i c)", p=P)
    out_flat = out_d.ap().rearrange("(p i) k -> p (i k)", p=P)  # [P, NI*KG]

    a_sb = io.tile([P, NI], F32)
    nc.sync.dma_start(a_sb[:], a_d.ap().rearrange("(p i) -> p i", p=P))

    # chunks: (i0, i1, blk0, blk1, has_rem)
    chunks = []
    b0 = 0
    while b0 < N_BLOCKS:
        b1 = min(b0 + CHUNK_BLOCKS, N_BLOCKS)
        last = b1 == N_BLOCKS
        chunks.append((9 * b0, NI if last else 9 * b1, b0, b1, last))
        b0 = b1
    copy_idx = 0
    for ci, (i0, i1, blk0, blk1, has_rem) in enumerate(chunks):
        ni = i1 - i0

        eyes_sb = io.tile([P, 3 * ni], F32, tag=f"eyes{ci}")
        nc.sync.dma_start(eyes_sb[:], eyes_f[:, 3 * i0 : 3 * i1])
        v_sb = io.tile([P, 3 * ni], F32, tag=f"v{ci}")
        nc.sync.dma_start(v_sb[:], v_f[:, 3 * i0 : 3 * i1])
        r_sb = io.tile([P, 9 * ni], F32, tag=f"r{ci}")
        nc.sync.dma_start(r_sb[:], r_f[:, 9 * i0 : 9 * i1])
        t_sb = io.tile([P, 3 * ni], F32, tag=f"t{ci}")
        nc.sync.dma_start(t_sb[:], t_f[:, 3 * i0 : 3 * i1])

        w2_sb = scr.tile([P, 3 * ni], F32, tag=f"w2_{ci}")
        d_aos = scr.tile([P, 3 * ni], F32, tag=f"d_{ci}")
        u_aos = scr.tile([P, 3 * ni], F32, tag=f"u_{ci}")
        g_aos = scr.tile([P, 6 * ni], F32, tag=f"g_{ci}")
        ag = scr.tile([P, 6 * ni], F32, tag=f"ag_{ci}")
        s_aos = scr.tile([P, 6 * ni + 16], F32, tag=f"s_{ci}")
        nv2 = scr.tile([P, ni], F32, tag=f"nv2_{ci}")
        rinv = scr.tile([P, ni], F32, tag=f"rinv_{ci}")

        v3 = v_sb[:].rearrange("p (i c) -> p i c", c=3)
        w23 = w2_sb[:].rearrange("p (i c) -> p i c", c=3)
        r9 = r_sb[:].rearrange("p (i a b) -> p i a b", a=3, b=3)
        d3 = d_aos[:].rearrange("p (i a) -> p i a", a=3)
        u3 = u_aos[:].rearrange("p (i a) -> p i a", a=3)
        g6 = g_aos[:].rearrange("p (i g) -> p i g", g=6)
        ag6 = ag[:].rearrange("p (i g) -> p i g", g=6)
        s6 = s_aos[:, 0 : 6 * ni].rearrange("p (i c) -> p i c", c=6)
        w0 = r9[:, :, :, 0]
        w1 = r9[:, :, :, 1]

        nc.vector.tensor_sub(w2_sb[:], t_sb[:], eyes_sb[:])

        dots = [
            (v3, v3, nv2[:]),
            (v3, w0, d3[:, :, 0]),
            (v3, w1, d3[:, :, 1]),
            (v3, w23, d3[:, :, 2]),
            (w0, w0, g6[:, :, 0]),
            (w1, w1, g6[:, :, 1]),
            (w23, w23, g6[:, :, 2]),
            (w0, w1, g6[:, :, 3]),
            (w0, w23, g6[:, :, 4]),
            (w1, w23, g6[:, :, 5]),
        ]
        for a_v, b_v, o_v in dots:
            tmp = tmpp.tile([P, 3 * ni], F32, tag=f"dm{ci}")
            t3 = tmp[:].rearrange("p (i c) -> p i c", c=3)
            if SQUARES_ON_ACT and a_v is b_v:
                nc.scalar.square(t3, a_v)
            else:
                nc.vector.tensor_mul(t3, a_v, b_v)
            nc.vector.tensor_reduce(o_v, t3, axis=X, op=ADD)

        nrm = tmpp.tile([P, ni], F32, tag=f"nrm{ci}")
        nc.scalar.sqrt(nrm[:], nv2[:])
        nc.vector.tensor_scalar_max(nrm[:], nrm[:], 1e-14)
        nc.vector.reciprocal(rinv[:], nrm[:])

        r_b3 = rinv[:].unsqueeze(2).broadcast_to((P, ni, 3))
        nc.vector.tensor_mul(u3, d3, r_b3)

        nc.vector.tensor_mul(s6[:, :, 0:3], u3, u3)
        nc.vector.tensor_mul(s6[:, :, 3:6:2], u3[:, :, 0:2], u3[:, :, 1:3])
        nc.vector.tensor_mul(s6[:, :, 4], u3[:, :, 0], u3[:, :, 2])

        a_b6 = a_sb[:, i0:i1].unsqueeze(2).broadcast_to((P, ni, 6))
        nc.vector.tensor_mul(ag6, g6, a_b6)
        nc.vector.tensor_sub(s6, s6, ag6)

        s_h = s_aos[:].tensor
        s_w = s_aos[:].ap[0][0]  # per-partition alloc width (elements)
        for b in range(blk0, blk1):
            lo = 54 * b - 6 * i0  # local col offset in this chunk's s_aos
            pack96 = pkp.tile([P, 96], F32, tag="pk")
            pack_dst = pack96[:].rearrange("p (g c) -> p g c", c=32)
            pack_src = bass.AP(s_h, lo, [[s_w, P], [18, 3], [1, 32]])
            if PACK_ON_ACT:
                nc.scalar.copy(pack_dst, pack_src)
            else:
                nc.vector.tensor_copy(pack_dst, pack_src)
            st_ps = psq.tile([96, P], F32)
            nc.tensor.transpose(st_ps[:], pack96[:], id_sb[:])
            st_sb = stp.tile([96, P], F32R if USE_F32R else F32)
            if ST_ON_ACT:
                nc.scalar.copy(st_sb[:], st_ps[:])
            else:
                nc.vector.tensor_copy(st_sb[:], st_ps[:])

            stage = stagep.tile([P, 3 * 507], F32, tag="stage")
            for g in range(BLOCK_GROUPS):
                o_ps = psm.tile([P, 508], F32, tag="mmout")
                nc.tensor.matmul(
                    o_ps[:],
                    st_sb[32 * g : 32 * g + 18, :],
                    q96_sb[32 * g : 32 * g + 18, :],
                    start=True,
                    stop=True,
                )
                dst = stage[:, 507 * g : 507 * (g + 1)]
                if COPY_MOD and copy_idx % COPY_MOD == COPY_MOD - 1:
                    nc.scalar.copy(dst, o_ps[:, 0:507])
                else:
                    nc.vector.tensor_copy(dst, o_ps[:, 0:507])
                copy_idx += 1
            out_dma_eng = nc.scalar if OUT_DMA_ON_ACT else nc.sync
            out_dma_eng.dma_start(
                out_flat[:, 1521 * b : 1521 * (b + 1)], stage[:]
            )

        if has_rem and REM_I:
            lo = 54 * N_BLOCKS - 6 * i0
            st_ps = psq.tile([96, P], F32)
            nc.tensor.transpose(
                st_ps[0 : 6 * REM_I, :], s_aos[:, lo : lo + 6 * REM_I], id_sb[:]
            )
            st_sb = stp.tile([96, P], F32R if USE_F32R else F32)
            nc.vector.tensor_copy(st_sb[0 : 6 * REM_I, :], st_ps[0 : 6 * REM_I, :])
            o_ps = psm.tile([P, 508], F32, tag="mmout")
            nc.tensor.matmul(
                o_ps[:, 0 : KG * REM_I],
                st_sb[0 : 6 * REM_I, :],
                q12_sb[:],
                start=True,
                stop=True,
            )
            stage = stagep.tile([P, 3 * 507], F32, tag="stage")
            nc.vector.tensor_copy(stage[:, 0 : KG * REM_I], o_ps[:, 0 : KG * REM_I])
            nc.sync.dma_start(
                out_flat[:, 1521 * N_BLOCKS :], stage[:, 0 : KG * REM_I]
            )


_NC_CACHE = {}


def _get_nc(reps=1):
    if reps not in _NC_CACHE:
        _NC_CACHE[reps] = build_nc(reps)
    return _NC_CACHE[reps]


def make_in_maps(eyes, v, R, t, alpha):
    q96 = make_q96()
    q12 = make_q12()
    ident = np.eye(P, dtype=np.float32)
    eyes = np.ascontiguousarray(eyes, np.float32).reshape(N_CORES, BC, 3)
    v = np.ascontiguousarray(v, np.float32).reshape(N_CORES, BC, 3)
    R = np.ascontiguousarray(R, np.float32).reshape(N_CORES, BC, 3, 3)
    t = np.ascontiguousarray(t, np.float32).reshape(N_CORES, BC, 3)
    alpha = np.ascontiguousarray(alpha, np.float32).reshape(N_CORES, BC)
    return [
        {
            "eyes": eyes[c], "v": v[c], "R": R[c], "t": t[c], "alpha": alpha[c],
            "q96": q96, "q12": q12, "ident": ident,
        }
        for c in range(N_CORES)
    ]


def kernel(eyes, v, R, t, alpha):
    nc = _get_nc(1)
    in_maps = make_in_maps(eyes, v, R, t, alpha)
    res = run_bass_kernel_spmd(nc, in_maps, list(range(N_CORES)))
    out = np.concatenate([res.results[c]["out"] for c in range(N_CORES)], axis=0)
    return out.astype(np.float32, copy=False)

